# revision 10
# baseline (speedup 1.0000x reference)
"""MeshTokenizer Trainium2 kernel (8-core data parallel).

Reference computation (per mesh):
  codes[f, j, c] = discretize(vertices[faces[f, j], c])   # gather + quantize
  input_ids     = [-1, (9 codes + sep=128) x NF (last sep dropped), -1]
  attention_mask = ones
  recon_faces   = undiscretize(codes of face 0)

Strategy: pure data parallel over B=64 meshes -> 8 meshes per NeuronCore.
Per core:
  - discretize + pack vertex codes into a per-mesh table of 8192 int32
    (x + 128*y + 16384*z, exact in f32 arithmetic)
  - replicate each mesh's packed table across its group of 16 SBUF
    partitions (DRAM bounce)
  - build the "wrap" index layout ap_gather needs (tile[p, s] =
    faces_flat[16*s + p]) with an int16 cast + interleaving DMA +
    hardware transpose DMA
  - gpsimd.ap_gather in 16 chunks (3072 indices each, all 8 meshes in
    parallel across the 8 Q7 cores)
  - unpack packed codes to x,y,z int32 on DVE, fan out to codes /
    discrete_face_coords / input_ids with sliced DMAs
"""

import os
import sys
import threading

sys.path.insert(0, "/opt/trn_rl_repo")

import numpy as np

import concourse.bass as bass
import concourse.tile as tile
from concourse import bacc, mybir
import concourse.bass_utils as bass_utils

# ---------------------------------------------------------------- constants
B = 64
NV = 8192
NF = 16384
N_CORES = 8
MPC = B // N_CORES            # 8 meshes per core
SLOTS = NF * 3                # 49152 face-vertex slots per mesh
IDS_LEN = NF * 10 + 1         # 163841
MAGIC = 12582912.0            # 1.5 * 2**23: float32 round-to-nearest-even trick
CHUNKS = 16
CH_SLOTS = SLOTS // CHUNKS    # 3072 slots per gather call
CH_FACES = CH_SLOTS // 3      # 1024 faces per chunk
QS = 16                       # per-group partition slices
Q_SLOTS = CH_SLOTS // QS      # 192 slots per (chunk, q)
Q_FACES = CH_FACES // QS      # 64 faces per (chunk, q)

i32 = mybir.dt.int32
i16 = mybir.dt.int16
f32 = mybir.dt.float32

_cache_lock = threading.Lock()
_cached_nc = None


def _build():
    nc = bacc.Bacc("TRN2", target_bir_lowering=False, debug=False)

    # -------- DRAM parameters (per-core shapes; host reshapes views)
    vert = nc.dram_tensor("vertices", [128, MPC * NV * 3 // 128], f32,
                          kind="ExternalInput").ap()          # [128, 1536]
    faces = nc.dram_tensor("faces", [128, MPC * SLOTS // 128], i32,
                           kind="ExternalInput").ap()         # [128, 3072]
    ids = nc.dram_tensor("input_ids", [MPC, IDS_LEN], i32,
                         kind="ExternalOutput").ap()
    mask = nc.dram_tensor("attention_mask", [MPC, IDS_LEN], f32,
                          kind="ExternalOutput").ap()
    codes = nc.dram_tensor("codes", [MPC, SLOTS * 3], i32,
                           kind="ExternalOutput").ap()
    disc = nc.dram_tensor("discrete", [MPC, SLOTS * 3], i32,
                          kind="ExternalOutput").ap()
    recon = nc.dram_tensor("recon", [MPC, 9], f32,
                           kind="ExternalOutput").ap()

    # internal DRAM scratch
    scr_rep = nc.dram_tensor("scr_rep", [MPC, 16, NV], i32).ap()       # 4 MB
    scr_v = nc.dram_tensor("scr_v", [SLOTS // 16, 128], i16).ap()      # [3072, 128]

    with tile.TileContext(nc) as tc:
        with (
            tc.tile_pool(name="persist", bufs=1) as pp,
            tc.tile_pool(name="stage", bufs=1) as sp,
            tc.tile_pool(name="gather", bufs=2) as gp,
            tc.tile_pool(name="unpack", bufs=2) as up,
        ):
            # ---------------- phase A: tables --------------------------------
            V = sp.tile([128, MPC * NV * 3 // 128], f32)      # [128, 1536]
            nc.sync.dma_start(V[:], vert[:])

            # discretize: C = min(max(rne((v+1)*64 - 0.5), 0), 127)  (exact f32)
            C = sp.tile([128, MPC * NV * 3 // 128], f32)
            nc.vector.tensor_scalar(C[:], V[:], 1.0, 64.0,
                                    mybir.AluOpType.add, mybir.AluOpType.mult)
            nc.vector.tensor_scalar(C[:], C[:], 0.5, MAGIC,
                                    mybir.AluOpType.subtract, mybir.AluOpType.add)
            nc.vector.tensor_scalar(C[:], C[:], MAGIC, 0.0,
                                    mybir.AluOpType.subtract, mybir.AluOpType.max)
            nc.vector.tensor_scalar(C[:], C[:], 127.0, None,
                                    mybir.AluOpType.min)

            # pack: p = x + 128*y + 16384*z (exact below 2**21)
            NW = MPC * NV // 128                              # 512 words/partition
            Cv = C[:].rearrange("p (w c) -> p w c", c=3)
            P1 = sp.tile([128, NW], f32)
            P2 = sp.tile([128, NW], f32)
            nc.vector.tensor_scalar(P1[:], Cv[:, :, 1], 128.0, None,
                                    mybir.AluOpType.mult)
            nc.vector.tensor_tensor(P1[:], P1[:], Cv[:, :, 0],
                                    mybir.AluOpType.add)
            nc.vector.tensor_scalar(P2[:], Cv[:, :, 2], 16384.0, None,
                                    mybir.AluOpType.mult)
            nc.vector.tensor_tensor(P1[:], P1[:], P2[:], mybir.AluOpType.add)
            Pi = sp.tile([128, NW], i32)
            nc.vector.tensor_copy(Pi[:], P1[:])               # f32 -> i32 cast

            # replicate each mesh's packed table to its 16 partitions via a
            # 16x-replicated DRAM image (keeps every DMA AP rectangular)
            wps = []
            for r in range(16):
                wps.append(nc.sync.dma_start(scr_rep[:, r, :], Pi[:]))
            TBL = pp.tile([128, NV], i32)
            ld = nc.sync.dma_start(TBL[:], scr_rep[:])
            for w in wps:
                tile.add_dep_helper(ld.ins, w.ins, True, "table after rep write")

            # ---------------- phase B: wrap index layout ---------------------
            F16 = sp.tile([128, MPC * SLOTS // 128], i16)     # [128, 3072]
            Fin = sp.tile([128, MPC * SLOTS // 128], i32)
            nc.sync.dma_start(Fin[:], faces[:])
            nc.vector.tensor_copy(F16[:], Fin[:])             # i32 -> i16 cast

            # scatter to DRAM interleaved: V4[mm][pp][jj][j2]
            v4 = scr_v.rearrange("s c -> (s c)").rearrange(
                "(pp jj mm j2) -> mm pp jj j2", pp=16, jj=192, mm=MPC)
            wv = nc.sync.dma_start(v4, F16[:])
            # hardware transpose: W[16m+p, s] = faces_m[16s+p]
            W = pp.tile([128, MPC * SLOTS // 128], i16)
            tp = nc.sync.dma_start_transpose(W[:], scr_v[:])
            tile.add_dep_helper(tp.ins, wv.ins, True, "transpose after scatter")

            # constants
            ones = pp.tile([128, 1280], f32)
            nc.vector.memset(ones[:], 1.0)
            neg1 = pp.tile([128, 2], i32)
            nc.vector.memset(neg1[:], -1)

            # ---------------- static output patterns -------------------------
            # attention mask (all ones)
            for m in range(MPC):
                nc.sync.dma_start(
                    mask[m, 0:IDS_LEN - 1].rearrange("(p f) -> p f", p=128),
                    ones[:, 0:1280])
            nc.sync.dma_start(mask[:, IDS_LEN - 1], ones[0:MPC, 0])

            # ---------------- phase C: gather + unpack + writeout ------------
            # token staging: [face, 10] per partition, separator column baked
            EB = []
            for i in range(2):
                E = pp.tile([128, CH_FACES * 10], i32, tag=f"e{i}")
                nc.vector.memset(
                    E[:].rearrange("p (k t) -> p k t", t=10)[:, :, 9], 128)
                EB.append(E)

            RC = pp.tile([128, 9], i32)
            last_body = []
            for c in range(CHUNKS):
                G = gp.tile([128, CH_SLOTS], i32, tag="g")
                nc.gpsimd.ap_gather(
                    G[:], TBL[:], W[:, c * (CH_SLOTS // 16):(c + 1) * (CH_SLOTS // 16)],
                    channels=128, num_elems=NV, d=1, num_idxs=CH_SLOTS)

                E = EB[c % 2]
                Ev = E[:].rearrange("p (k t) -> p k t", t=10)
                Gv = G[:].rearrange("p (k v) -> p k v", v=3)
                # unpack packed codes into token positions 3v+c of each face
                nc.vector.tensor_scalar(
                    Ev[:, :, 0:9].rearrange("p k (v x) -> p k v x", x=3)[:, :, :, 0],
                    Gv[:], 127, None, mybir.AluOpType.bitwise_and)
                nc.vector.tensor_scalar(
                    Ev[:, :, 0:9].rearrange("p k (v x) -> p k v x", x=3)[:, :, :, 1],
                    Gv[:], 7, 127, mybir.AluOpType.arith_shift_right,
                    mybir.AluOpType.bitwise_and)
                nc.vector.tensor_scalar(
                    Ev[:, :, 0:9].rearrange("p k (v x) -> p k v x", x=3)[:, :, :, 2],
                    Gv[:], 14, None, mybir.AluOpType.arith_shift_right)

                if c == 0:
                    nc.vector.tensor_copy(RC[:], E[:, 0:9])

                # fan out: each of the 16 partitions in a group ships 1/16
                for q in range(QS):
                    eng = nc.sync
                    srcE = E[q::16, q * Q_FACES * 10:(q + 1) * Q_FACES * 10]
                    srcEv = srcE.rearrange("p (k t) -> p k t", t=10)
                    # codes / discrete: 9 tokens per face (skip sep column)
                    base = c * CH_SLOTS * 3 + q * Q_SLOTS * 3
                    cv = codes[:, base:base + Q_SLOTS * 3] \
                        .rearrange("m (k t) -> m k t", t=9)
                    dv = disc[:, base:base + Q_SLOTS * 3] \
                        .rearrange("m (k t) -> m k t", t=9)
                    eng.dma_start(cv, srcEv[:, :, 0:9])
                    eng.dma_start(dv, srcEv[:, :, 0:9])
                    # input_ids body: tokens + separator, contiguous, offset 1
                    fbase = c * CH_FACES + q * Q_FACES
                    b = eng.dma_start(
                        ids[:, 1 + fbase * 10: 1 + (fbase + Q_FACES) * 10],
                        srcE)
                    if c == CHUNKS - 1:
                        last_body.append(b)

            # edge -1 tokens; ids[*, 163840] overwrites the final separator
            e0 = nc.sync.dma_start(ids[:, 0:1], neg1[0:MPC, 0:1])
            e1 = nc.sync.dma_start(ids[:, IDS_LEN - 1:IDS_LEN],
                                   neg1[0:MPC, 0:1])
            for b in last_body:
                tile.add_dep_helper(e1.ins, b.ins, True, "edge after body")

            # recon: (c + 0.5)/64 - 1 on face-0 codes
            RF = pp.tile([128, 9], f32)
            nc.vector.tensor_copy(RF[:], RC[:])               # i32 -> f32 exact
            nc.vector.tensor_scalar(RF[:], RF[:], 1.0 / 64.0, 2.0 ** -7 - 1.0,
                                    mybir.AluOpType.mult, mybir.AluOpType.add)
            nc.sync.dma_start(recon[:, :], RF[0:128:16, :])

    nc.compile()
    return nc


def _get_nc():
    global _cached_nc
    with _cache_lock:
        if _cached_nc is None:
            _cached_nc = _build()
    return _cached_nc


def kernel(vertices: np.ndarray, faces: np.ndarray):
    vertices = np.ascontiguousarray(vertices, dtype=np.float32)
    faces = np.ascontiguousarray(faces, dtype=np.int32)
    assert vertices.shape == (B, NV, 3) and faces.shape == (B, NF, 3)

    nc = _get_nc()
    in_maps = []
    for core in range(N_CORES):
        v = vertices[core * MPC:(core + 1) * MPC].reshape(128, -1)
        f = faces[core * MPC:(core + 1) * MPC].reshape(128, -1)
        in_maps.append({"vertices": v, "faces": f})

    res = bass_utils.run_bass_kernel_spmd(nc, in_maps, list(range(N_CORES)))
    kernel.last_results = res

    ids = np.concatenate([r["input_ids"] for r in res.results], axis=0)
    mask = np.concatenate([r["attention_mask"] for r in res.results], axis=0)
    codes = np.concatenate(
        [r["codes"].reshape(MPC, NF, 3, 3) for r in res.results], axis=0)
    recon = np.concatenate(
        [r["recon"].reshape(MPC, 1, 3, 3) for r in res.results], axis=0)
    return ids, mask, codes, codes.copy(), recon


# revision 14
# speedup vs baseline: 1.2180x; 1.2180x over previous
"""MeshTokenizer Trainium2 kernel (8-core data parallel).

Reference computation (per mesh):
  codes[f, j, c] = discretize(vertices[faces[f, j], c])   # gather + quantize
  input_ids     = [-1, (9 codes + sep=128) x NF (last sep dropped), -1]
  attention_mask = ones
  recon_faces   = undiscretize(codes of face 0)

Strategy: pure data parallel over B=64 meshes -> 8 meshes per NeuronCore.
Per core:
  - discretize + pack vertex codes into a per-mesh table of 8192 int32
    (x + 128*y + 16384*z, exact in f32 arithmetic)
  - replicate each mesh's packed table across its group of 16 SBUF
    partitions (DRAM bounce)
  - build the "wrap" index layout ap_gather needs (tile[p, s] =
    faces_flat[16*s + p]) with an int16 cast + interleaving DMA +
    hardware transpose DMA
  - gpsimd.ap_gather in 16 chunks (3072 indices each, all 8 meshes in
    parallel across the 8 Q7 cores)
  - unpack packed codes to x,y,z int32 on DVE, fan out to codes /
    discrete_face_coords / input_ids with sliced DMAs
"""

import os
import sys
import threading

sys.path.insert(0, "/opt/trn_rl_repo")

import numpy as np

import concourse.bass as bass
import concourse.tile as tile
from concourse import bacc, mybir
import concourse.bass_utils as bass_utils

# ---------------------------------------------------------------- constants
B = 64
NV = 8192
NF = 16384
N_CORES = 8
MPC = B // N_CORES            # 8 meshes per core
SLOTS = NF * 3                # 49152 face-vertex slots per mesh
IDS_LEN = NF * 10 + 1         # 163841
MAGIC = 12582912.0            # 1.5 * 2**23: float32 round-to-nearest-even trick
CHUNKS = 16
CH_SLOTS = SLOTS // CHUNKS    # 3072 slots per gather call
CH_FACES = CH_SLOTS // 3      # 1024 faces per chunk
QS = 16                       # per-group partition slices
Q_SLOTS = CH_SLOTS // QS      # 192 slots per (chunk, q)
Q_FACES = CH_FACES // QS      # 64 faces per (chunk, q)

i32 = mybir.dt.int32
i16 = mybir.dt.int16
f32 = mybir.dt.float32

_cache_lock = threading.Lock()
_cached_nc = None


def _build():
    nc = bacc.Bacc("TRN2", target_bir_lowering=False, debug=False)

    # -------- DRAM parameters (per-core shapes; host reshapes views)
    vert = nc.dram_tensor("vertices", [128, MPC * NV * 3 // 128], f32,
                          kind="ExternalInput").ap()          # [128, 1536]
    faces = nc.dram_tensor("faces", [128, MPC * SLOTS // 128], i32,
                           kind="ExternalInput").ap()         # [128, 3072]
    ids = nc.dram_tensor("input_ids", [MPC, IDS_LEN], i32,
                         kind="ExternalOutput").ap()
    mask = nc.dram_tensor("attention_mask", [MPC, IDS_LEN], f32,
                          kind="ExternalOutput").ap()
    codes = nc.dram_tensor("codes", [MPC, SLOTS * 3], i32,
                           kind="ExternalOutput").ap()
    recon = nc.dram_tensor("recon", [MPC, 9], f32,
                           kind="ExternalOutput").ap()

    # internal DRAM scratch
    scr_rep = nc.dram_tensor("scr_rep", [MPC, 16, NV], i32).ap()       # 4 MB
    scr_v = nc.dram_tensor("scr_v", [SLOTS // 16, 128], i16).ap()      # [3072, 128]

    from contextlib import ExitStack
    with tile.TileContext(nc) as tc:
        with (
            tc.tile_pool(name="persist", bufs=1) as pp,
            tc.tile_pool(name="gather", bufs=2) as gp,
            tc.tile_pool(name="unpack", bufs=2) as up,
        ):
            _stage_ctx = ExitStack()
            sp = _stage_ctx.enter_context(tc.tile_pool(name="stage", bufs=1))
            # ---------------- phase A: tables --------------------------------
            V = sp.tile([128, MPC * NV * 3 // 128], f32)      # [128, 1536]
            nc.sync.dma_start(V[:], vert[:])

            # discretize: C = min(max(rne((v+1)*64 - 0.5), 0), 127)  (exact f32)
            C = V
            nc.vector.tensor_scalar(C[:], V[:], 1.0, 64.0,
                                    mybir.AluOpType.add, mybir.AluOpType.mult)
            nc.vector.tensor_scalar(C[:], C[:], 0.5, MAGIC,
                                    mybir.AluOpType.subtract, mybir.AluOpType.add)
            nc.vector.tensor_scalar(C[:], C[:], MAGIC, 0.0,
                                    mybir.AluOpType.subtract, mybir.AluOpType.max)
            nc.vector.tensor_scalar(C[:], C[:], 127.0, None,
                                    mybir.AluOpType.min)

            # pack: p = x + 128*y + 16384*z (exact below 2**21)
            NW = MPC * NV // 128                              # 512 words/partition
            Cv = C[:].rearrange("p (w c) -> p w c", c=3)
            P1 = sp.tile([128, NW], f32)
            P2 = sp.tile([128, NW], f32)
            nc.vector.tensor_scalar(P1[:], Cv[:, :, 1], 128.0, None,
                                    mybir.AluOpType.mult)
            nc.vector.tensor_tensor(P1[:], P1[:], Cv[:, :, 0],
                                    mybir.AluOpType.add)
            nc.vector.tensor_scalar(P2[:], Cv[:, :, 2], 16384.0, None,
                                    mybir.AluOpType.mult)
            nc.vector.tensor_tensor(P1[:], P1[:], P2[:], mybir.AluOpType.add)
            Pi = sp.tile([128, NW], i32)
            nc.vector.tensor_copy(Pi[:], P1[:])               # f32 -> i32 cast

            # replicate each mesh's packed table to its 16 partitions via a
            # 16x-replicated DRAM image (keeps every DMA AP rectangular)
            wps = []
            for r in range(16):
                wps.append(nc.sync.dma_start(scr_rep[:, r, :], Pi[:]))
            TBL = pp.tile([128, NV], i32)
            ld = nc.sync.dma_start(TBL[:], scr_rep[:])
            for w in wps:
                tile.add_dep_helper(ld.ins, w.ins, True, "table after rep write")

            # ---------------- phase B: wrap index layout ---------------------
            F16 = sp.tile([128, MPC * SLOTS // 128], i16, tag="P1")
            Fin = sp.tile([128, MPC * SLOTS // 128], i32, tag="V")
            nc.sync.dma_start(Fin[:], faces[:])
            nc.vector.tensor_copy(F16[:], Fin[:])             # i32 -> i16 cast

            # scatter to DRAM interleaved: V4[mm][pp][jj][j2]
            v4 = scr_v.rearrange("s c -> (s c)").rearrange(
                "(pp jj mm j2) -> mm pp jj j2", pp=16, jj=192, mm=MPC)
            wv = nc.sync.dma_start(v4, F16[:])
            # hardware transpose: W[16m+p, s] = faces_m[16s+p]
            W = pp.tile([128, MPC * SLOTS // 128], i16)
            tp = nc.sync.dma_start_transpose(W[:], scr_v[:])
            tile.add_dep_helper(tp.ins, wv.ins, True, "transpose after scatter")

            _stage_ctx.close()   # release phase-A staging SBUF

            # constants
            ones = pp.tile([128, 1280], f32)
            nc.vector.memset(ones[:], 1.0)
            neg1 = pp.tile([128, 2], i32)
            nc.vector.memset(neg1[:], -1)

            # ---------------- static output patterns -------------------------
            # attention mask (all ones)
            for m in range(MPC):
                nc.sync.dma_start(
                    mask[m, 0:IDS_LEN - 1].rearrange("(p f) -> p f", p=128),
                    ones[:, 0:1280])
            nc.sync.dma_start(mask[:, IDS_LEN - 1], ones[0:MPC, 0])

            # ---------------- phase C: gather + unpack + writeout ------------
            # token staging: [face, 10] per partition, separator column baked
            E = pp.tile([128, CH_FACES * 10], i32, tag="e0")
            nc.vector.memset(
                E[:].rearrange("p (k t) -> p k t", t=10)[:, :, 9], 128)

            NQ = 8                       # partition slices per group used
            US = CH_SLOTS * 3 // NQ      # 1152 i32 codes slice per q
            ES = CH_FACES * 10 // NQ     # 1280 i32 ids slice per q
            RC = pp.tile([128, 9], i32)
            last_body = []
            for c in range(CHUNKS):
                G = gp.tile([128, CH_SLOTS], i32, tag="g")
                nc.gpsimd.ap_gather(
                    G[:], TBL[:], W[:, c * (CH_SLOTS // 16):(c + 1) * (CH_SLOTS // 16)],
                    channels=128, num_elems=NV, d=1, num_idxs=CH_SLOTS)

                # dense unpack -> U (for codes)
                U = up.tile([128, CH_SLOTS * 3], i32, tag="u")
                Uv = U[:].rearrange("p (s c) -> p s c", c=3)
                nc.vector.tensor_scalar(Uv[:, :, 0], G[:], 127, None,
                                        mybir.AluOpType.bitwise_and)
                nc.vector.tensor_scalar(Uv[:, :, 1], G[:], 7, 127,
                                        mybir.AluOpType.arith_shift_right,
                                        mybir.AluOpType.bitwise_and)
                nc.vector.tensor_scalar(Uv[:, :, 2], G[:], 14, None,
                                        mybir.AluOpType.arith_shift_right)

                # strided unpack -> E (for input_ids, separator column baked)
                Ev = E[:].rearrange("p (k t) -> p k t", t=10)
                Et = Ev[:, :, 0:9].rearrange("p k (v x) -> p k v x", x=3)
                Gv = G[:].rearrange("p (k v) -> p k v", v=3)
                nc.vector.tensor_scalar(Et[:, :, :, 0], Gv[:], 127, None,
                                        mybir.AluOpType.bitwise_and)
                nc.vector.tensor_scalar(Et[:, :, :, 1], Gv[:], 7, 127,
                                        mybir.AluOpType.arith_shift_right,
                                        mybir.AluOpType.bitwise_and)
                nc.vector.tensor_scalar(Et[:, :, :, 2], Gv[:], 14, None,
                                        mybir.AluOpType.arith_shift_right)

                if c == 0:
                    nc.vector.tensor_copy(RC[:], U[:, 0:9])

                # fan out: 8 partitions per group each ship 1/8 of the chunk
                for q in range(NQ):
                    base = c * CH_SLOTS * 3 + q * US
                    nc.sync.dma_start(codes[:, base:base + US],
                                      U[q::16, q * US:(q + 1) * US])
                    fbase = (c * CH_FACES * 10 + q * ES)
                    b = nc.scalar.dma_start(
                        ids[:, 1 + fbase: 1 + fbase + ES],
                        E[q::16, q * ES:(q + 1) * ES])
                    if c == CHUNKS - 1:
                        last_body.append(b)

            # edge -1 tokens; ids[*, 163840] overwrites the final separator
            e0 = nc.sync.dma_start(ids[:, 0:1], neg1[0:MPC, 0:1])
            e1 = nc.sync.dma_start(ids[:, IDS_LEN - 1:IDS_LEN],
                                   neg1[0:MPC, 0:1])
            for b in last_body:
                tile.add_dep_helper(e1.ins, b.ins, True, "edge after body")

            # recon: (c + 0.5)/64 - 1 on face-0 codes
            RF = pp.tile([128, 9], f32)
            nc.vector.tensor_copy(RF[:], RC[:])               # i32 -> f32 exact
            nc.vector.tensor_scalar(RF[:], RF[:], 1.0 / 64.0, 2.0 ** -7 - 1.0,
                                    mybir.AluOpType.mult, mybir.AluOpType.add)
            nc.sync.dma_start(recon[:, :], RF[0:128:16, :])

    nc.compile()
    return nc


def _get_nc():
    global _cached_nc
    with _cache_lock:
        if _cached_nc is None:
            _cached_nc = _build()
    return _cached_nc


def kernel(vertices: np.ndarray, faces: np.ndarray):
    vertices = np.ascontiguousarray(vertices, dtype=np.float32)
    faces = np.ascontiguousarray(faces, dtype=np.int32)
    assert vertices.shape == (B, NV, 3) and faces.shape == (B, NF, 3)

    nc = _get_nc()
    in_maps = []
    for core in range(N_CORES):
        v = vertices[core * MPC:(core + 1) * MPC].reshape(128, -1)
        f = faces[core * MPC:(core + 1) * MPC].reshape(128, -1)
        in_maps.append({"vertices": v, "faces": f})

    res = bass_utils.run_bass_kernel_spmd(nc, in_maps, list(range(N_CORES)))
    kernel.last_results = res

    ids = np.concatenate([r["input_ids"] for r in res.results], axis=0)
    mask = np.concatenate([r["attention_mask"] for r in res.results], axis=0)
    codes = np.concatenate(
        [r["codes"].reshape(MPC, NF, 3, 3) for r in res.results], axis=0)
    recon = np.concatenate(
        [r["recon"].reshape(MPC, 1, 3, 3) for r in res.results], axis=0)
    return ids, mask, codes, codes.copy(), recon


# revision 15
# speedup vs baseline: 1.3516x; 1.1097x over previous
"""MeshTokenizer Trainium2 kernel (8-core data parallel).

Reference computation (per mesh):
  codes[f, j, c] = discretize(vertices[faces[f, j], c])   # gather + quantize
  input_ids     = [-1, (9 codes + sep=128) x NF (last sep dropped), -1]
  attention_mask = ones
  recon_faces   = undiscretize(codes of face 0)

Strategy: pure data parallel over B=64 meshes -> 8 meshes per NeuronCore.
Per core:
  - discretize + pack vertex codes into a per-mesh table of 8192 int32
    (x + 128*y + 16384*z, exact in f32 arithmetic)
  - replicate each mesh's packed table across its group of 16 SBUF
    partitions (DRAM bounce)
  - build the "wrap" index layout ap_gather needs (tile[p, s] =
    faces_flat[16*s + p]) with an int16 cast + interleaving DMA +
    hardware transpose DMA
  - gpsimd.ap_gather in 16 chunks (3072 indices each, all 8 meshes in
    parallel across the 8 Q7 cores)
  - unpack packed codes to x,y,z int32 on DVE, fan out to codes /
    discrete_face_coords / input_ids with sliced DMAs
"""

import os
import sys
import threading

sys.path.insert(0, "/opt/trn_rl_repo")

import numpy as np

import concourse.bass as bass
import concourse.tile as tile
from concourse import bacc, mybir
import concourse.bass_utils as bass_utils

# ---------------------------------------------------------------- constants
B = 64
NV = 8192
NF = 16384
N_CORES = 8
MPC = B // N_CORES            # 8 meshes per core
SLOTS = NF * 3                # 49152 face-vertex slots per mesh
IDS_LEN = NF * 10 + 1         # 163841
MAGIC = 12582912.0            # 1.5 * 2**23: float32 round-to-nearest-even trick
CHUNKS = 32
CH_SLOTS = SLOTS // CHUNKS    # 3072 slots per gather call
CH_FACES = CH_SLOTS // 3      # 1024 faces per chunk
QS = 16                       # per-group partition slices
Q_SLOTS = CH_SLOTS // QS      # 192 slots per (chunk, q)
Q_FACES = CH_FACES // QS      # 64 faces per (chunk, q)

i32 = mybir.dt.int32
i16 = mybir.dt.int16
f32 = mybir.dt.float32

_cache_lock = threading.Lock()
_cached_nc = None


def _build():
    nc = bacc.Bacc("TRN2", target_bir_lowering=False, debug=False)

    # -------- DRAM parameters (per-core shapes; host reshapes views)
    vert = nc.dram_tensor("vertices", [128, MPC * NV * 3 // 128], f32,
                          kind="ExternalInput").ap()          # [128, 1536]
    faces = nc.dram_tensor("faces", [128, MPC * SLOTS // 128], i32,
                           kind="ExternalInput").ap()         # [128, 3072]
    ids = nc.dram_tensor("input_ids", [MPC, IDS_LEN], i32,
                         kind="ExternalOutput").ap()
    mask = nc.dram_tensor("attention_mask", [MPC, IDS_LEN], f32,
                          kind="ExternalOutput").ap()
    codes = nc.dram_tensor("codes", [MPC, SLOTS * 3], i32,
                           kind="ExternalOutput").ap()
    recon = nc.dram_tensor("recon", [MPC, 9], f32,
                           kind="ExternalOutput").ap()

    # internal DRAM scratch
    scr_rep = nc.dram_tensor("scr_rep", [MPC, 16, NV], i32).ap()       # 4 MB
    scr_v = nc.dram_tensor("scr_v", [SLOTS // 16, 128], i16).ap()      # [3072, 128]

    from contextlib import ExitStack
    with tile.TileContext(nc) as tc:
        with (
            tc.tile_pool(name="persist", bufs=1) as pp,
            tc.tile_pool(name="gather", bufs=2) as gp,
            tc.tile_pool(name="unpack", bufs=2) as up,
        ):
            _stage_ctx = ExitStack()
            sp = _stage_ctx.enter_context(tc.tile_pool(name="stage", bufs=1))
            # ---------------- phase A: tables --------------------------------
            V = sp.tile([128, MPC * NV * 3 // 128], f32)      # [128, 1536]
            nc.sync.dma_start(V[:], vert[:])

            # discretize: C = min(max(rne((v+1)*64 - 0.5), 0), 127)  (exact f32)
            C = V
            nc.vector.tensor_scalar(C[:], V[:], 1.0, 64.0,
                                    mybir.AluOpType.add, mybir.AluOpType.mult)
            nc.vector.tensor_scalar(C[:], C[:], 0.5, MAGIC,
                                    mybir.AluOpType.subtract, mybir.AluOpType.add)
            nc.vector.tensor_scalar(C[:], C[:], MAGIC, 0.0,
                                    mybir.AluOpType.subtract, mybir.AluOpType.max)
            nc.vector.tensor_scalar(C[:], C[:], 127.0, None,
                                    mybir.AluOpType.min)

            # pack: p = x + 128*y + 16384*z (exact below 2**21)
            NW = MPC * NV // 128                              # 512 words/partition
            Cv = C[:].rearrange("p (w c) -> p w c", c=3)
            P1 = sp.tile([128, NW], f32)
            P2 = sp.tile([128, NW], f32)
            nc.vector.tensor_scalar(P1[:], Cv[:, :, 1], 128.0, None,
                                    mybir.AluOpType.mult)
            nc.vector.tensor_tensor(P1[:], P1[:], Cv[:, :, 0],
                                    mybir.AluOpType.add)
            nc.vector.tensor_scalar(P2[:], Cv[:, :, 2], 16384.0, None,
                                    mybir.AluOpType.mult)
            nc.vector.tensor_tensor(P1[:], P1[:], P2[:], mybir.AluOpType.add)
            Pi = sp.tile([128, NW], i32)
            nc.vector.tensor_copy(Pi[:], P1[:])               # f32 -> i32 cast

            # replicate each mesh's packed table to its 16 partitions via a
            # 16x-replicated DRAM image (keeps every DMA AP rectangular)
            wps = []
            for r in range(16):
                wps.append(nc.sync.dma_start(scr_rep[:, r, :], Pi[:]))
            TBL = pp.tile([128, NV], i32)
            ld = nc.sync.dma_start(TBL[:], scr_rep[:])
            for w in wps:
                tile.add_dep_helper(ld.ins, w.ins, True, "table after rep write")

            # ---------------- phase B: wrap index layout ---------------------
            F16 = sp.tile([128, MPC * SLOTS // 128], i16, tag="P1")
            Fin = sp.tile([128, MPC * SLOTS // 128], i32, tag="V")
            nc.sync.dma_start(Fin[:], faces[:])
            nc.vector.tensor_copy(F16[:], Fin[:])             # i32 -> i16 cast

            # scatter to DRAM interleaved: V4[mm][pp][jj][j2]
            v4 = scr_v.rearrange("s c -> (s c)").rearrange(
                "(pp jj mm j2) -> mm pp jj j2", pp=16, jj=192, mm=MPC)
            wv = nc.sync.dma_start(v4, F16[:])
            # hardware transpose: W[16m+p, s] = faces_m[16s+p]
            W = pp.tile([128, MPC * SLOTS // 128], i16)
            tp = nc.sync.dma_start_transpose(W[:], scr_v[:])
            tile.add_dep_helper(tp.ins, wv.ins, True, "transpose after scatter")

            _stage_ctx.close()   # release phase-A staging SBUF

            # constants
            ones = pp.tile([128, 1280], f32)
            nc.vector.memset(ones[:], 1.0)
            neg1 = pp.tile([128, 2], i32)
            nc.vector.memset(neg1[:], -1)

            # ---------------- static output patterns -------------------------
            # attention mask (all ones)
            for m in range(MPC):
                nc.sync.dma_start(
                    mask[m, 0:IDS_LEN - 1].rearrange("(p f) -> p f", p=128),
                    ones[:, 0:1280])
            nc.sync.dma_start(mask[:, IDS_LEN - 1], ones[0:MPC, 0])

            # ---------------- phase C: gather + unpack + writeout ------------
            # token staging: [face, 10] per partition, separator column baked
            EB = []
            for i in range(2):
                Ei = pp.tile([128, CH_FACES * 10], i32, tag=f"e{i}")
                nc.vector.memset(
                    Ei[:].rearrange("p (k t) -> p k t", t=10)[:, :, 9], 128)
                EB.append(Ei)

            NQ = 8                       # partition slices per group used
            US = CH_SLOTS * 3 // NQ      # 1152 i32 codes slice per q
            ES = CH_FACES * 10 // NQ     # 1280 i32 ids slice per q
            RC = pp.tile([128, 9], i32)
            last_body = []
            for c in range(CHUNKS):
                G = gp.tile([128, CH_SLOTS], i32, tag="g")
                nc.gpsimd.ap_gather(
                    G[:], TBL[:], W[:, c * (CH_SLOTS // 16):(c + 1) * (CH_SLOTS // 16)],
                    channels=128, num_elems=NV, d=1, num_idxs=CH_SLOTS)

                # dense unpack -> U (for codes)
                U = up.tile([128, CH_SLOTS * 3], i32, tag="u")
                Uv = U[:].rearrange("p (s c) -> p s c", c=3)
                nc.vector.tensor_scalar(Uv[:, :, 0], G[:], 127, None,
                                        mybir.AluOpType.bitwise_and)
                nc.vector.tensor_scalar(Uv[:, :, 1], G[:], 7, 127,
                                        mybir.AluOpType.arith_shift_right,
                                        mybir.AluOpType.bitwise_and)
                nc.vector.tensor_scalar(Uv[:, :, 2], G[:], 14, None,
                                        mybir.AluOpType.arith_shift_right)

                # strided unpack -> E (for input_ids, separator column baked)
                E = EB[c % 2]
                Ev = E[:].rearrange("p (k t) -> p k t", t=10)
                Et = Ev[:, :, 0:9].rearrange("p k (v x) -> p k v x", x=3)
                Gv = G[:].rearrange("p (k v) -> p k v", v=3)
                nc.vector.tensor_scalar(Et[:, :, :, 0], Gv[:], 127, None,
                                        mybir.AluOpType.bitwise_and)
                nc.vector.tensor_scalar(Et[:, :, :, 1], Gv[:], 7, 127,
                                        mybir.AluOpType.arith_shift_right,
                                        mybir.AluOpType.bitwise_and)
                nc.vector.tensor_scalar(Et[:, :, :, 2], Gv[:], 14, None,
                                        mybir.AluOpType.arith_shift_right)

                if c == 0:
                    nc.vector.tensor_copy(RC[:], U[:, 0:9])

                # fan out: 8 partitions per group each ship 1/8 of the chunk
                for q in range(NQ):
                    base = c * CH_SLOTS * 3 + q * US
                    nc.sync.dma_start(codes[:, base:base + US],
                                      U[q::16, q * US:(q + 1) * US])
                    fbase = (c * CH_FACES * 10 + q * ES)
                    b = nc.scalar.dma_start(
                        ids[:, 1 + fbase: 1 + fbase + ES],
                        E[q::16, q * ES:(q + 1) * ES])
                    if c == CHUNKS - 1:
                        last_body.append(b)

            # edge -1 tokens; ids[*, 163840] overwrites the final separator
            e0 = nc.sync.dma_start(ids[:, 0:1], neg1[0:MPC, 0:1])
            e1 = nc.sync.dma_start(ids[:, IDS_LEN - 1:IDS_LEN],
                                   neg1[0:MPC, 0:1])
            for b in last_body:
                tile.add_dep_helper(e1.ins, b.ins, True, "edge after body")

            # recon: (c + 0.5)/64 - 1 on face-0 codes
            RF = pp.tile([128, 9], f32)
            nc.vector.tensor_copy(RF[:], RC[:])               # i32 -> f32 exact
            nc.vector.tensor_scalar(RF[:], RF[:], 1.0 / 64.0, 2.0 ** -7 - 1.0,
                                    mybir.AluOpType.mult, mybir.AluOpType.add)
            nc.sync.dma_start(recon[:, :], RF[0:128:16, :])

    nc.compile()
    return nc


def _get_nc():
    global _cached_nc
    with _cache_lock:
        if _cached_nc is None:
            _cached_nc = _build()
    return _cached_nc


def kernel(vertices: np.ndarray, faces: np.ndarray):
    vertices = np.ascontiguousarray(vertices, dtype=np.float32)
    faces = np.ascontiguousarray(faces, dtype=np.int32)
    assert vertices.shape == (B, NV, 3) and faces.shape == (B, NF, 3)

    nc = _get_nc()
    in_maps = []
    for core in range(N_CORES):
        v = vertices[core * MPC:(core + 1) * MPC].reshape(128, -1)
        f = faces[core * MPC:(core + 1) * MPC].reshape(128, -1)
        in_maps.append({"vertices": v, "faces": f})

    res = bass_utils.run_bass_kernel_spmd(nc, in_maps, list(range(N_CORES)))
    kernel.last_results = res

    ids = np.concatenate([r["input_ids"] for r in res.results], axis=0)
    mask = np.concatenate([r["attention_mask"] for r in res.results], axis=0)
    codes = np.concatenate(
        [r["codes"].reshape(MPC, NF, 3, 3) for r in res.results], axis=0)
    recon = np.concatenate(
        [r["recon"].reshape(MPC, 1, 3, 3) for r in res.results], axis=0)
    return ids, mask, codes, codes.copy(), recon
